# revision 7
# baseline (speedup 1.0000x reference)
"""Linear-chain CRF Viterbi decode on Trainium2 (Bass/Tile), 8-core data parallel.

Algorithm (bitwise match to the f32 jax reference after a host-side tie fixup):
  forward:  alpha_0 = emit_0;  alpha_t[j] = max_i(alpha_{t-1}[i] + T[i,j]) + emit_t[j]
            Engine split: DVE materializes all scores with single f32 adds
            (tensor_tensor, two ping-pong groups of 4 row-tiles); GPSIMD does
            the grouped max-reduce and the emit add.  f32 add/max on either
            engine is IEEE-exact, so alphas match the reference bitwise.
            alphas for every t are spilled to a DRAM scratch buffer.
  backward: tag_T = argmax(alpha_T); tag_t = argmax_i(alpha_t[i] + T[i, tag_{t+1}]).
            Per step the PE computes s = alpha + T[:, tag] directly in PSUM:
            matmul(idn, alpha) writes alpha (products x*1/x*0 are exact), then
            matmul(h^T, blockdiag(T^T)) accumulates the gathered column
            (one-hot products, exact).  DVE does reduce_max + is_equal; the
            is_equal output IS the one-hot f32 output row.  GPSIMD scatters it
            into the output chunk.
  ties:     is_equal marks ALL argmax positions; exact f32 score ties (~133
            rows out of 4.2M on the reference input) yield multi-hot cells.
            kernel() detects rows containing any non-one-hot cell (sum != 1)
            and recomputes those rows exactly on the host (first-argmax,
            matching jnp.argmax).  Output is then bitwise-exact.

Sharding: pure batch data-parallelism, batch 8192 -> 8 cores x 1024 rows.
"""

import numpy as np
from contextlib import ExitStack

B = 8192
T = 512
K = 24
NCORES = 8
BL = B // NCORES          # rows per core
P = 128                   # partitions

_prog_cache = {}


def _bview(sl, dims):
    """AP with custom free dims (incl. step-0 broadcast), keeping the slice's
    offset and partition pair."""
    from concourse.ap import AP
    return AP(sl.tensor, sl.offset, [list(sl.ap[0])] + [list(d) for d in dims])


def _dview(ap, offset, dims):
    """Arbitrary strided view of a DRAM tensor ([[step,count],...], elem offset)."""
    from concourse.ap import AP
    return AP(ap.tensor, offset, [list(d) for d in dims])


def _split_excess_waits(nc):
    """Walrus rejects engine instructions whose embedded sync struct carries
    more than one semaphore wait ("Too many sync wait commands").  Engine
    instruction streams execute in order, so moving excess waits onto
    ENGINE_NOPs inserted immediately before the instruction is semantically
    identical.  Sequencer-only instructions (SP/DMA) use standalone wait
    commands and are exempt."""
    from concourse import mybir
    eng_map = {
        mybir.EngineType.DVE: nc.vector,
        mybir.EngineType.Activation: nc.scalar,
        mybir.EngineType.PE: nc.tensor,
        mybir.EngineType.Pool: nc.gpsimd,
    }
    ctr = 0
    for f in nc.m.functions:
        for blk in f.blocks:
            changed = False
            out = []
            for ins in blk.instructions:
                si = ins.sync_info
                if si is not None and len(si.on_wait) > 1:
                    for w in si.on_wait:
                        if ins.engine in eng_map:
                            # Drain = benign sequencer-side stall, accepted as
                            # a wait carrier on every compute engine.
                            nop = mybir.InstDrain(name=f"WD-{ctr}", ins=[],
                                                  outs=[])
                        else:
                            nop = mybir.InstNoOp(name=f"WS-{ctr}", ins=[],
                                                 outs=[])
                        ctr += 1
                        nop.engine = ins.engine
                        nop.sync_info = mybir.SyncInfo(on_wait=[w],
                                                       on_update=[])
                        out.append(nop)
                    ins.sync_info = mybir.SyncInfo(
                        on_wait=[], on_update=list(si.on_update))
                    changed = True
                out.append(ins)
            if changed:
                blk.instructions = out
    return ctr


def build_program(BLc, Tc, CH, nq=2, demit=False, split_waits=True, repeat=1,
                  phases="both"):
    """Build the per-core Bass program.

    BLc: local batch rows; Tc: sequence length; CH: time-chunk size.
    Forward: 2 ping-pong groups of NT/2 row-tiles; per group DVE does the
    score adds for the first GT-nq tiles and the grouped max-reduce for the
    whole group, GPSIMD (Pool) does the score adds for the last nq tiles and
    the emit add (free-dim reduce is DVE-only on TRN2 stock ucode).
    """
    import concourse.bass as bass
    import concourse.tile as tile
    from concourse import mybir

    f32 = mybir.dt.float32
    Alu = mybir.AluOpType
    X = mybir.AxisListType.X

    NT = BLc // P             # batch tiles per core (8)
    NG = 2                    # ping-pong groups
    GT = NT // NG             # tiles per group (4)
    NCH = Tc // CH            # time chunks
    CK = CH * K               # free elems per (tile, chunk)
    GCK = GT * CK             # per-group free elems per chunk
    GK = GT * K               # group width (96)
    GKK = GT * K * K          # score elems per group (2304)

    nc = bass.Bass("TRN2", target_bir_lowering=False, debug=False)

    inp = nc.dram_tensor("inp", [BLc, Tc, K], f32, kind="ExternalInput").ap()
    tb_d = nc.dram_tensor("tbcast", [P, K * K], f32, kind="ExternalInput").ap()
    bd_d = nc.dram_tensor("blockdiag", [GK, GK], f32, kind="ExternalInput").ap()
    idn_d = nc.dram_tensor("idn", [P, P], f32, kind="ExternalInput").ap()
    outp = nc.dram_tensor("outp", [BLc, Tc, K], f32, kind="ExternalOutput").ap()
    adram = nc.dram_tensor("alpha_scr", [NT, P, Tc, K], f32, kind="Internal").ap()

    with tile.TileContext(nc) as tc, ExitStack() as ctx:
        const = ctx.enter_context(tc.tile_pool(name="const", bufs=1))

        tb = const.tile([P, K * K], f32)
        nc.sync.dma_start(tb[:, :], tb_d)
        bd = const.tile([GK, GK], f32)
        nc.sync.dma_start(bd[:, :], bd_d)
        idn = const.tile([P, P], f32)
        nc.sync.dma_start(idn[:, :], idn_d)

        for _rep in range(repeat):
            if _rep:
                tc.strict_bb_all_engine_barrier()
            # ---------------- forward ----------------
            if phases in ("both", "fwd"):
              if isinstance(nq, (tuple, list)) and len(nq) == 2 and \
                      isinstance(nq[0], (tuple, list)):
                  gts, nqs = [list(v) for v in nq]
              elif isinstance(nq, (tuple, list)):
                  gts, nqs = [GT] * NG, list(nq)
              else:
                  gts, nqs = [GT] * NG, [nq] * NG
              g0s = [sum(gts[:g]) for g in range(NG)]
              fctx = ctx.enter_context(ExitStack())
              femis = fctx.enter_context(tc.tile_pool(name="femis", bufs=2))
              fhist = fctx.enter_context(tc.tile_pool(name="fhist", bufs=2))
              fs = fctx.enter_context(tc.tile_pool(name="fs", bufs=2))
              fm = fctx.enter_context(tc.tile_pool(name="fm", bufs=2))
              prev_hist = [None] * NG
              for ch in range(NCH):
                  emis = femis.tile([P, NT * CK], f32, tag="emis")
                  nc.sync.dma_start(
                      emis[:, :].rearrange("p (n c) -> p n c", c=CK),
                      _dview(inp, ch * CK,
                             [[Tc * K, P], [P * Tc * K, NT], [1, CK]]))
                  hists = [fhist.tile([P, gts[g] * CK], f32, tag=f"hist{g}",
                                      name=f"hist{g}")
                           for g in range(NG)]
                  for t_c in range(CH):
                      t = ch * CH + t_c
                      for g in range(NG):
                          gt = gts[g]
                          h_sl = _bview(hists[g][:, t_c * K:t_c * K + 1],
                                        [[CK, gt], [1, K]])
                          e_sl = _bview(
                              emis[:, g0s[g] * CK + t_c * K:
                                   g0s[g] * CK + t_c * K + 1],
                              [[CK, gt], [1, K]])
                          if t == 0:
                              nc.gpsimd.tensor_copy(h_sl, e_sl)
                              continue
                          if t_c == 0:
                              prev = prev_hist[g]
                          else:
                              prev = hists[g]
                          pt = (CH - 1) if t_c == 0 else (t_c - 1)
                          # scores: s[p, n, j, i] = T[i, j] + alpha[p, n, i]
                          s = fs.tile([P, gt * K * K], f32, tag=f"s{g}",
                                      name=f"s{g}")
                          nqg = nqs[g]
                          ndt = gt - nqg
                          if ndt > 0:
                              nc.vector.tensor_tensor(
                                  _bview(s[:, 0:1],
                                         [[K * K, ndt], [K, K], [1, K]]),
                                  _bview(tb[:, 0:1], [[0, ndt], [K, K], [1, K]]),
                                  _bview(prev[:, pt * K:pt * K + 1],
                                         [[CK, ndt], [0, K], [1, K]]),
                                  op=Alu.add)
                          if nqg > 0:
                              nc.gpsimd.tensor_tensor(
                                  _bview(s[:, ndt * K * K:ndt * K * K + 1],
                                         [[K * K, nqg], [K, K], [1, K]]),
                                  _bview(tb[:, 0:1], [[0, nqg], [K, K], [1, K]]),
                                  _bview(prev[:, ndt * CK + pt * K:
                                              ndt * CK + pt * K + 1],
                                         [[CK, nqg], [0, K], [1, K]]),
                                  op=Alu.add)
                          m = fm.tile([P, gt * K], f32, tag=f"m{g}",
                                      name=f"m{g}")
                          nc.vector.tensor_reduce(
                              _bview(m[:, 0:1], [[K, gt], [1, K]]),
                              _bview(s[:, 0:1], [[K * K, gt], [K, K], [1, K]]),
                              axis=X, op=Alu.max)
                          emit_eng = nc.vector if demit else nc.gpsimd
                          emit_eng.tensor_tensor(
                              h_sl, _bview(m[:, 0:1], [[K, gt], [1, K]]),
                              e_sl, op=Alu.add)
                  for g in range(NG):
                      nc.sync.dma_start(
                          _dview(adram, g0s[g] * P * Tc * K + ch * CK,
                                 [[Tc * K, P], [P * Tc * K, gts[g]], [1, CK]]),
                          hists[g][:, :].rearrange("p (n c) -> p n c", c=CK))
                  prev_hist = hists
              fctx.close()
            tc.strict_bb_all_engine_barrier()

            # ---------------- backward (traceback) ----------------
            if phases in ("both", "bwd"):
              bctx = ctx.enter_context(ExitStack())
              ta = bctx.enter_context(tc.tile_pool(name="ta", bufs=2))
              to = bctx.enter_context(tc.tile_pool(name="to", bufs=2))
              tsm = bctx.enter_context(tc.tile_pool(name="tsm", bufs=2))
              tmx = bctx.enter_context(tc.tile_pool(name="tmx", bufs=2))
              th = bctx.enter_context(tc.tile_pool(name="th", bufs=3))
              tps = bctx.enter_context(tc.tile_pool(name="tps", bufs=2,
                                                    space="PSUM"))
              h_list = [None] * NG
              for ch in range(NCH - 1, -1, -1):
                  ach = ta.tile([P, NT * CK], f32, tag="ach")
                  nc.sync.dma_start(
                      ach[:, :].rearrange("p (n c) -> p n c", c=CK),
                      _dview(adram, ch * CK,
                             [[Tc * K, P], [P * Tc * K, NT], [1, CK]]))
                  och = to.tile([P, NT * CK], f32, tag="och")
                  for t_c in range(CH - 1, -1, -1):
                      t = ch * CH + t_c
                      for g in range(NG):
                          a_sl = _bview(
                              ach[:, g * GCK + t_c * K:g * GCK + t_c * K + 1],
                              [[CK, GT], [1, K]])
                          h_new = th.tile([P, GK], f32, tag=f"h{g}")
                          mx = tmx.tile([P, GT], f32, tag=f"mx{g}")
                          if t == Tc - 1:
                              nc.vector.tensor_reduce(
                                  _bview(mx[:, 0:1], [[1, GT]]), a_sl,
                                  axis=X, op=Alu.max)
                              nc.vector.tensor_tensor(
                                  _bview(h_new[:, 0:1], [[K, GT], [1, K]]),
                                  a_sl,
                                  _bview(mx[:, 0:1], [[1, GT], [0, K]]),
                                  op=Alu.is_equal)
                          else:
                              # gather+add on PE: gp = alpha + T[:, tag_{t+1}]
                              # (mm_alpha must be TensorE so has_written bits
                              # allow mm_gather's start=False accumulate).
                              htp = tps.tile([GK, P], f32, tag=f"htp{g}")
                              nc.tensor.transpose(
                                  htp[:, :], h_list[g][:, :], idn[:, :])
                              hts = tsm.tile([GK, P], f32, tag=f"hts{g}")
                              nc.scalar.copy(hts[:, :], htp[:, :])
                              gp = tps.tile([P, GK], f32, tag=f"gp{g}")
                              nc.tensor.matmul(gp[:, :], idn[:, :], a_sl,
                                               start=True, stop=False)
                              nc.tensor.matmul(gp[:, :], hts[:, :], bd[:, :],
                                               start=False, stop=True)
                              nc.vector.tensor_reduce(
                                  _bview(mx[:, 0:1], [[1, GT]]),
                                  _bview(gp[:, 0:1], [[K, GT], [1, K]]),
                                  axis=X, op=Alu.max)
                              nc.vector.tensor_tensor(
                                  _bview(h_new[:, 0:1], [[K, GT], [1, K]]),
                                  _bview(gp[:, 0:1], [[K, GT], [1, K]]),
                                  _bview(mx[:, 0:1], [[1, GT], [0, K]]),
                                  op=Alu.is_equal)
                          h_list[g] = h_new
                          nc.gpsimd.tensor_copy(
                              _bview(och[:, g * GCK + t_c * K:
                                         g * GCK + t_c * K + 1],
                                     [[CK, GT], [1, K]]),
                              _bview(h_new[:, 0:1], [[K, GT], [1, K]]))
                  nc.sync.dma_start(
                      _dview(outp, ch * CK,
                             [[Tc * K, P], [P * Tc * K, NT], [1, CK]]),
                      och[:, :].rearrange("p (n c) -> p n c", c=CK))
              bctx.close()
    if split_waits:
        _split_excess_waits(nc)
    return nc


def make_aux(transitions, BLc):
    """Host-side constant tensors derived from the transitions matrix."""
    NT = BLc // P
    GT = NT // 2
    GK = GT * K
    Tm = np.asarray(transitions, dtype=np.float32)
    tb = np.ascontiguousarray(
        np.broadcast_to(Tm.T.reshape(1, K * K), (P, K * K))).astype(np.float32)
    bdm = np.zeros((GK, GK), np.float32)
    for g in range(GT):
        bdm[g * K:(g + 1) * K, g * K:(g + 1) * K] = Tm.T
    idn = np.eye(P, dtype=np.float32)
    return {"tbcast": tb, "blockdiag": bdm, "idn": idn}


def _fix_tie_rows(out, inputs, transitions):
    """Exact host-side repair of rows whose on-chip traceback hit a score tie.

    is_equal marks every argmax position, so a tie produces a multi-hot cell
    (row-sum != 1) at the tie step; everything decoded before that step in the
    row is then suspect.  Recompute those rows with numpy f32 Viterbi
    (np.argmax = first index, matching jnp.argmax)."""
    sums = out.sum(axis=2)
    bad = np.unique(np.nonzero(np.abs(sums - 1.0) > 0.5)[0])
    if len(bad) == 0:
        return out
    x = np.asarray(inputs[bad], dtype=np.float32)      # [nb, T, K]
    Tm = np.asarray(transitions, dtype=np.float32)
    nb, Tt, Kk = x.shape
    alphas = np.empty((Tt, nb, Kk), np.float32)
    bps = np.empty((Tt, nb, Kk), np.int64)
    alpha = x[:, 0].copy()
    alphas[0] = alpha
    for t in range(1, Tt):
        s = alpha[:, :, None] + Tm[None]               # [nb, K(i), K(j)] f32
        bps[t] = s.argmax(axis=1)
        alpha = s.max(axis=1).astype(np.float32) + x[:, t]
        alphas[t] = alpha
    tags = np.empty((Tt, nb), np.int64)
    tag = alphas[Tt - 1].argmax(axis=1)
    tags[Tt - 1] = tag
    for t in range(Tt - 2, -1, -1):
        s = (alphas[t] + Tm[:, tag].T).astype(np.float32)
        tag = s.argmax(axis=1)
        tags[t] = tag
    oh = np.zeros((nb, Tt, Kk), np.float32)
    ii, tt = np.meshgrid(np.arange(nb), np.arange(Tt), indexing="ij")
    oh[ii, tt, tags.T] = 1.0
    out[bad] = oh
    return out


def run(inputs, transitions, trace=False, **spmd_kwargs):
    from concourse.bass_utils import run_bass_kernel_spmd

    key = (BL, T)
    if key not in _prog_cache:
        _prog_cache[key] = build_program(BL, T, CH=32)
    nc = _prog_cache[key]

    inputs = np.asarray(inputs, dtype=np.float32)
    aux = make_aux(transitions, BL)
    in_maps = [
        {"inp": np.ascontiguousarray(inputs[c * BL:(c + 1) * BL]), **aux}
        for c in range(NCORES)
    ]
    res = run_bass_kernel_spmd(nc, in_maps, core_ids=list(range(NCORES)),
                               trace=trace, **spmd_kwargs)
    out = np.concatenate([r["outp"] for r in res.results], axis=0)
    out = np.ascontiguousarray(out, dtype=np.float32)
    out = _fix_tie_rows(out, inputs, transitions)
    return out, res


def kernel(inputs, transitions):
    out, _ = run(inputs, transitions)
    return out

